# revision 4
# baseline (speedup 1.0000x reference)
"""BiDAF-style attention kernel for Trainium2, 8-core data-parallel over batch.

Problem (per batch b):
  sim[c,q] = ctx[c]@w_c + qry[q]@w_q + sum_h ctx[c,h] w_m[h] qry[q,h] + att_b
  alpha = softmax_q(sim);        a[c] = sum_q alpha[c,q] qry[q]
  beta  = softmax_c(max_q sim);  bv   = sum_c beta[c] ctx[c]
  out = [ctx | a | ctx*a | ctx*bv]          (C, 4H)

Layout: sim computed as [c(part), q(free)] tiles of [128, 129]; the 129th
column carries ctx@w_c (augmented rhs column), so the per-row bias for the
exp comes out of the same matmul. qvec broadcast enters via a K=1 rank-1
matmul. Softmaxes never subtract the max (inputs are O(10), exp is safe in
f32, and the shift cancels mathematically) and exp row-sums come from the
scalar engine's accum_out.
"""

import numpy as np

import concourse.bass as bass
import concourse.tile as tile
from concourse import mybir
from concourse.bass_utils import run_bass_kernel_spmd
from concourse.masks import make_identity

B, C, Q, H = 64, 1024, 128, 256
NCORES = 8
BL = B // NCORES          # batches per core
CT = C // 128             # context row-tiles per batch
F32 = mybir.dt.float32
F32R = mybir.dt.float32r

# fp32r (4-byte matmul fast path) on the N>=256 matmuls: 4x fewer PE cycles.
USE_FP32R = False


def _r(ap):
    return ap.bitcast(F32R) if USE_FP32R else ap


def split_waits(nc, max_waits=1):
    """walrus codegen in this container rejects >1 sem wait per instruction;
    move excess waits onto same-engine NoOps inserted just before."""
    n_new = 0
    for f in nc.m.functions:
        for blk in f.blocks:
            out = []
            for ins in blk.instructions:
                waits = list(ins.sync_info.on_wait) if ins.sync_info else []
                if len(waits) > max_waits:
                    extra, keep = waits[:-max_waits], waits[-max_waits:]
                    for j in range(0, len(extra), max_waits):
                        nop = mybir.InstNoOp(name=f"I-wsplit-{n_new}", ins=[], outs=[])
                        n_new += 1
                        nop.engine = ins.engine
                        nop.sync_info = mybir.SyncInfo(
                            on_wait=list(extra[j : j + max_waits]), on_update=[]
                        )
                        out.append(nop)
                    ins.sync_info.on_wait = list(keep)
                out.append(ins)
            blk.instructions = out
    return n_new


def build():
    nc = bass.Bass()
    ctx_d = nc.dram_tensor("context", [BL, C, H], F32, kind="ExternalInput")
    q_d = nc.dram_tensor("query", [BL, Q, H], F32, kind="ExternalInput")
    w_d = nc.dram_tensor("att_w", [3 * H], F32, kind="ExternalInput")
    b_d = nc.dram_tensor("att_b", [1], F32, kind="ExternalInput")
    out_d = nc.dram_tensor("out", [BL, C, 4 * H], F32, kind="ExternalOutput")

    X = mybir.AxisListType.X
    MUL = mybir.AluOpType.mult
    ADD = mybir.AluOpType.add
    EXP = mybir.ActivationFunctionType.Exp

    with tile.TileContext(nc) as tc:
        from contextlib import ExitStack

        with ExitStack() as ctx:
            consts = ctx.enter_context(tc.tile_pool(name="consts", bufs=1))
            ctxp = ctx.enter_context(tc.tile_pool(name="ctx", bufs=2))
            ctxTp = ctx.enter_context(tc.tile_pool(name="ctxT", bufs=2))
            qp = ctx.enter_context(tc.tile_pool(name="qp", bufs=2))
            esp = ctx.enter_context(tc.tile_pool(name="es", bufs=3))
            stagp = ctx.enter_context(tc.tile_pool(name="stag", bufs=4))
            cbvp = ctx.enter_context(tc.tile_pool(name="cbv", bufs=4))
            smallp = ctx.enter_context(tc.tile_pool(name="small", bufs=8))
            ps_sim = ctx.enter_context(tc.tile_pool(name="ps_sim", bufs=2, space="PSUM"))
            ps_tp = ctx.enter_context(tc.tile_pool(name="ps_tp", bufs=2, space="PSUM"))
            ps_a = ctx.enter_context(tc.tile_pool(name="ps_a", bufs=2, space="PSUM"))
            ps_bb = ctx.enter_context(tc.tile_pool(name="ps_bb", bufs=1, space="PSUM"))
            ps_sm = ctx.enter_context(tc.tile_pool(name="ps_sm", bufs=1, space="PSUM"))

            ident = consts.tile([128, 128], F32)
            make_identity(nc, ident[:, :])
            ones_row = consts.tile([1, 128], F32)
            nc.vector.memset(ones_row[:, :], 1.0)
            ones_col = consts.tile([128, 1], F32)
            nc.vector.memset(ones_col[:, :], 1.0)
            # att_w as 6 columns: [w_c h0|h1, w_q h0|h1, w_m h0|h1]
            wcols = consts.tile([128, 6], F32)
            nc.gpsimd.dma_start(
                out=wcols[:, :],
                in_=bass.AP(tensor=w_d, offset=0, ap=[[1, 128], [128, 6]]),
            )
            # w_q broadcast across partitions for the qvec row-reduction
            wqb = consts.tile([128, H], F32)
            nc.gpsimd.dma_start(
                out=wqb[:, :],
                in_=bass.AP(tensor=w_d, offset=H, ap=[[0, 128], [1, H]]),
            )
            attb = consts.tile([128, 1], F32)
            nc.gpsimd.dma_start(
                out=attb[:, :],
                in_=bass.AP(tensor=b_d, offset=0, ap=[[0, 128], [1, 1]]),
            )

            for b in range(BL):
                ctx_sb = ctxp.tile([128, CT, H], F32)
                nc.sync.dma_start(
                    out=ctx_sb[:, :, :],
                    in_=ctx_d[b].rearrange("(ct p) h -> p ct h", p=128),
                )
                q_sb = qp.tile([128, H], F32)
                nc.sync.dma_start(out=q_sb[:, :], in_=q_d[b])

                # qT, scaled by w_m, with w_c appended as col 128
                rhs_aug = qp.tile([128, 2, 129], F32)
                for ht in range(2):
                    tp = ps_tp.tile([128, 128], F32)
                    nc.tensor.transpose(
                        tp[:, :], q_sb[:, ht * 128 : (ht + 1) * 128], ident[:, :]
                    )
                    nc.vector.tensor_scalar_mul(
                        rhs_aug[:, ht, 0:128], tp[:, :], wcols[:, 4 + ht : 5 + ht]
                    )
                    nc.vector.tensor_copy(
                        rhs_aug[:, ht, 128:129], wcols[:, ht : ht + 1]
                    )

                # qvec[q] = qry[q] @ w_q  (row-reduce on DVE, then PE-transpose
                # the column into a row for the rank-1 broadcast matmul)
                scr = qp.tile([128, H], F32)
                qvec_col = smallp.tile([128, 1], F32)
                nc.vector.tensor_mul(scr[:, :], q_sb[:, :], wqb[:, :])
                nc.vector.reduce_sum(qvec_col[:, :], scr[:, :], axis=X)
                qvp = ps_sm.tile([1, 128], F32, tag="sm")
                nc.tensor.transpose(qvp[:, :], qvec_col[:, :], ident[:, :])
                qvec_row = smallp.tile([1, 128], F32)
                nc.vector.tensor_copy(qvec_row[:, :], qvp[:, :])

                # ctxT[h, c] per h-half
                ctxT = ctxTp.tile([128, 2, C], F32)
                for ht in range(2):
                    for ct in range(CT):
                        tp = ps_tp.tile([128, 128], F32)
                        nc.tensor.transpose(
                            tp[:, :],
                            ctx_sb[:, ct, ht * 128 : (ht + 1) * 128],
                            ident[:, :],
                        )
                        nc.scalar.copy(ctxT[:, ht, ct * 128 : (ct + 1) * 128], tp[:, :])

                M8 = smallp.tile([128, CT], F32)
                for ct in range(CT):
                    sim = ps_sim.tile([128, 129], F32)
                    nc.tensor.matmul(
                        sim[:, :],
                        lhsT=ctxT[:, 0, ct * 128 : (ct + 1) * 128],
                        rhs=rhs_aug[:, 0, :],
                        start=True,
                        stop=False,
                    )
                    nc.tensor.matmul(
                        sim[:, 0:128],
                        lhsT=ones_row[:, :],
                        rhs=qvec_row[:, :],
                        start=False,
                        stop=False,
                        skip_group_check=True,
                    )
                    nc.tensor.matmul(
                        sim[:, :],
                        lhsT=ctxT[:, 1, ct * 128 : (ct + 1) * 128],
                        rhs=rhs_aug[:, 1, :],
                        start=False,
                        stop=True,
                    )
                    # exp(sim + cvec + att_b) with row-sums for free
                    cvecb = smallp.tile([128, 1], F32)
                    nc.scalar.add(cvecb[:, :], sim[:, 128:129], attb[:, 0:1])
                    es = esp.tile([128, 128], F32)
                    S_col = smallp.tile([128, 1], F32)
                    nc.scalar.activation(
                        out=es[:, :],
                        in_=sim[:, 0:128],
                        func=EXP,
                        bias=cvecb[:, 0:1],
                        scale=1.0,
                        accum_out=S_col[:, :],
                    )
                    nc.vector.reduce_max(M8[:, ct : ct + 1], es[:, :], axis=X)
                    esTp = ps_tp.tile([128, 128], F32, tag="tp")
                    nc.tensor.transpose(esTp[:, :], es[:, :], ident[:, :])
                    esT = esp.tile([128, 128], F32)
                    nc.scalar.copy(esT[:, :], esTp[:, :])
                    a_ps = ps_a.tile([128, H], F32)
                    nc.tensor.matmul(
                        a_ps[:, :], lhsT=_r(esT[:, :]), rhs=_r(q_sb[:, :]),
                        start=True, stop=True,
                    )
                    rS = smallp.tile([128, 1], F32)
                    nc.vector.reciprocal(rS[:, :], S_col[:, :])
                    stag = stagp.tile([128, 2, H], F32)
                    nc.vector.tensor_scalar_mul(stag[:, 0, :], a_ps[:, :], rS[:, :])
                    nc.vector.tensor_mul(
                        stag[:, 1, :], ctx_sb[:, ct, :], stag[:, 0, :]
                    )
                    r0, r1 = ct * 128, (ct + 1) * 128
                    nc.sync.dma_start(out=out_d[b, r0:r1, 0:H], in_=ctx_sb[:, ct, :])
                    nc.sync.dma_start(out=out_d[b, r0:r1, H : 3 * H], in_=stag[:, :, :])

                # beta path: M8 col ct = rowmax of exp(sim) = exp(rowmax sim)
                Sb = ps_sm.tile([1, CT], F32, tag="sm")
                nc.tensor.matmul(
                    Sb[:, :], lhsT=ones_col[:, :], rhs=M8[:, :], start=True, stop=True
                )
                Sb1 = smallp.tile([1, 1], F32)
                nc.vector.reduce_sum(Sb1[:, :], Sb[:, :], axis=X)
                rSb = smallp.tile([1, 1], F32)
                nc.vector.reciprocal(rSb[:, :], Sb1[:, :])
                bv_ps = ps_sm.tile([1, H], F32, tag="sm")
                for ct in range(CT):
                    nc.tensor.matmul(
                        bv_ps[:, :],
                        lhsT=_r(M8[:, ct : ct + 1]),
                        rhs=_r(ctx_sb[:, ct, :]),
                        start=(ct == 0),
                        stop=(ct == CT - 1),
                    )
                bv = smallp.tile([1, H], F32)
                nc.vector.tensor_scalar_mul(bv[:, :], bv_ps[:, :], rSb[:, :])
                bb_ps = ps_bb.tile([128, H], F32)
                nc.tensor.matmul(
                    bb_ps[:, :], lhsT=ones_row[:, :], rhs=bv[:, :], start=True, stop=True
                )
                for ct in range(CT):
                    cbv = cbvp.tile([128, H], F32)
                    nc.vector.tensor_mul(cbv[:, :], ctx_sb[:, ct, :], bb_ps[:, :])
                    nc.sync.dma_start(
                        out=out_d[b, ct * 128 : (ct + 1) * 128, 3 * H : 4 * H],
                        in_=cbv[:, :],
                    )

    split_waits(nc)
    return nc


_NC = None
LAST_RESULT = None


def kernel(_trace=False, **inputs):
    global _NC, LAST_RESULT
    if _NC is None:
        _NC = build()
    context = np.ascontiguousarray(np.asarray(inputs["context"], dtype=np.float32))
    query = np.ascontiguousarray(np.asarray(inputs["query"], dtype=np.float32))
    att_w = np.ascontiguousarray(np.asarray(inputs["att_w"], dtype=np.float32))
    att_b = np.asarray(inputs["att_b"], dtype=np.float32).reshape(1)
    in_maps = [
        {
            "context": np.ascontiguousarray(context[i * BL : (i + 1) * BL]),
            "query": np.ascontiguousarray(query[i * BL : (i + 1) * BL]),
            "att_w": att_w,
            "att_b": att_b,
        }
        for i in range(NCORES)
    ]
    res = run_bass_kernel_spmd(
        _NC, in_maps, core_ids=list(range(NCORES)), trace=_trace
    )
    LAST_RESULT = res
    return np.concatenate([r["out"] for r in res.results], axis=0)


# revision 7
# speedup vs baseline: 1.0923x; 1.0923x over previous
"""BiDAF-style attention kernel for Trainium2, 8-core data-parallel over batch.

Problem (per batch b):
  sim[c,q] = ctx[c]@w_c + qry[q]@w_q + sum_h ctx[c,h] w_m[h] qry[q,h] + att_b
  alpha = softmax_q(sim);        a[c] = sum_q alpha[c,q] qry[q]
  beta  = softmax_c(max_q sim);  bv   = sum_c beta[c] ctx[c]
  out = [ctx | a | ctx*a | ctx*bv]          (C, 4H)

Key algebra:
  - ctx@w_c (cvec) is constant along q -> cancels in the alpha softmax and in
    a; it only shifts the beta logits. So sim' = sim - cvec is computed on the
    PE and cvec enters only as a tiny per-c weight exp(cvec) on the beta path.
  - att_b is a global constant -> cancels everywhere; dropped entirely.
  - No max subtraction inside softmax: logits are O(10), exp is safe in f32,
    and the shift cancels exactly.
  - max_q exp(sim') = exp(max_q sim'), so the beta max is the rowmax of the
    already-computed exp values.

Layout: sim' is built TRANSPOSED, simT [q=128 part, c=1024 free], so the main
matmuls run N=512/257 with fp32r (1 cycle/row, single pass):
  simT = qTs_r^T @ ctxT_r   (qTs = w_m * qT); qvec = qry@w_q is a
  per-partition scalar in this layout and enters via the exp bias for free.
  expsimT = exp(simT + qvec) written as f32r -> directly the lhsT of the
  a-matmul: [a | S] = expsimT^T @ [qry | 1], S = alpha normalizer from the
  ones column. The beta max comes from PE-transposing expsimT tiles back to
  [c,q] and DVE row-maxing them straight out of PSUM.
"""

import numpy as np

import concourse.bass as bass
import concourse.tile as tile
from concourse import mybir
from concourse.bass_utils import run_bass_kernel_spmd
from concourse.masks import make_identity

B, C, Q, H = 64, 1024, 128, 256
NCORES = 8
BL = B // NCORES          # batches per core
CT = C // 128             # context row-tiles per batch
F32 = mybir.dt.float32
F32R = mybir.dt.float32r


def split_waits(nc, max_waits=1):
    """walrus codegen in this container rejects >1 sem wait per instruction;
    move excess waits onto same-engine NoOps inserted just before."""
    n_new = 0
    for f in nc.m.functions:
        for blk in f.blocks:
            out = []
            for ins in blk.instructions:
                waits = list(ins.sync_info.on_wait) if ins.sync_info else []
                if len(waits) > max_waits:
                    extra, keep = waits[:-max_waits], waits[-max_waits:]
                    for j in range(0, len(extra), max_waits):
                        nop = mybir.InstNoOp(name=f"I-wsplit-{n_new}", ins=[], outs=[])
                        n_new += 1
                        nop.engine = ins.engine
                        nop.sync_info = mybir.SyncInfo(
                            on_wait=list(extra[j : j + max_waits]), on_update=[]
                        )
                        out.append(nop)
                    ins.sync_info.on_wait = list(keep)
                out.append(ins)
            blk.instructions = out
    return n_new


def build():
    nc = bass.Bass()
    ctx_d = nc.dram_tensor("context", [BL, C, H], F32, kind="ExternalInput")
    q_d = nc.dram_tensor("query", [BL, Q, H], F32, kind="ExternalInput")
    w_d = nc.dram_tensor("att_w", [3 * H], F32, kind="ExternalInput")
    b_d = nc.dram_tensor("att_b", [1], F32, kind="ExternalInput")
    out_d = nc.dram_tensor("out", [BL, C, 4 * H], F32, kind="ExternalOutput")

    X = mybir.AxisListType.X
    EXP = mybir.ActivationFunctionType.Exp

    with tile.TileContext(nc) as tc:
        from contextlib import ExitStack

        with ExitStack() as ctx:
            consts = ctx.enter_context(tc.tile_pool(name="consts", bufs=1))
            ctxp = ctx.enter_context(tc.tile_pool(name="ctx", bufs=2))
            ctxTp = ctx.enter_context(tc.tile_pool(name="ctxT", bufs=2))
            qp = ctx.enter_context(tc.tile_pool(name="qp", bufs=2))
            esp = ctx.enter_context(tc.tile_pool(name="es", bufs=2))
            stagp = ctx.enter_context(tc.tile_pool(name="stag", bufs=4))
            cbvp = ctx.enter_context(tc.tile_pool(name="cbv", bufs=4))
            smallp = ctx.enter_context(tc.tile_pool(name="small", bufs=8))
            ps_sim = ctx.enter_context(tc.tile_pool(name="ps_sim", bufs=1, space="PSUM"))
            ps_tp = ctx.enter_context(tc.tile_pool(name="ps_tp", bufs=2, space="PSUM"))
            ps_a = ctx.enter_context(tc.tile_pool(name="ps_a", bufs=2, space="PSUM"))
            ps_bb = ctx.enter_context(tc.tile_pool(name="ps_bb", bufs=1, space="PSUM"))
            ps_sm = ctx.enter_context(tc.tile_pool(name="ps_sm", bufs=1, space="PSUM"))

            ident = consts.tile([128, 128], F32)
            make_identity(nc, ident[:, :])
            ident_r = consts.tile([128, 128], F32R)
            nc.vector.tensor_copy(ident_r[:, :], ident[:, :])
            ones_col = consts.tile([128, 1], F32)
            nc.vector.memset(ones_col[:, :], 1.0)
            ones_row = consts.tile([1, 128], F32)
            nc.vector.memset(ones_row[:, :], 1.0)
            ones_row_r = consts.tile([1, 128], F32R)
            nc.vector.tensor_copy(ones_row_r[:, :], ones_row[:, :])
            # att_w as 6 columns: [w_c h0|h1, w_q h0|h1, w_m h0|h1]
            wcols = consts.tile([128, 6], F32)
            nc.gpsimd.dma_start(
                out=wcols[:, :],
                in_=bass.AP(tensor=w_d, offset=0, ap=[[1, 128], [128, 6]]),
            )
            # w_q broadcast across partitions for the qvec row-reduction
            wqb = consts.tile([128, H], F32)
            nc.gpsimd.dma_start(
                out=wqb[:, :],
                in_=bass.AP(tensor=w_d, offset=H, ap=[[0, 128], [1, H]]),
            )

            for b in range(BL):
                ctx_sb = ctxp.tile([128, CT, H], F32)
                nc.sync.dma_start(
                    out=ctx_sb[:, :, :],
                    in_=ctx_d[b].rearrange("(ct p) h -> p ct h", p=128),
                )
                q_sb = qp.tile([128, H], F32)
                nc.sync.dma_start(out=q_sb[:, :], in_=q_d[b])

                # qT scaled by w_m -> lhsT of the simT matmul (f32r)
                qTs_r = qp.tile([128, 2, 128], F32R)
                for ht in range(2):
                    tp = ps_tp.tile([128, 128], F32, tag="tp")
                    nc.tensor.transpose(
                        tp[:, :], q_sb[:, ht * 128 : (ht + 1) * 128], ident[:, :]
                    )
                    nc.vector.tensor_scalar_mul(
                        qTs_r[:, ht, :], tp[:, :], wcols[:, 4 + ht : 5 + ht]
                    )

                # qvec[q] = qry[q] @ w_q as a column (q = partition dim)
                scr = qp.tile([128, H], F32)
                qvec_col = smallp.tile([128, 1], F32)
                nc.vector.tensor_mul(scr[:, :], q_sb[:, :], wqb[:, :])
                nc.vector.reduce_sum(qvec_col[:, :], scr[:, :], axis=X)

                # rhs of the a-matmul: [qry | 1] rounded to f32r
                qaug_r = qp.tile([128, H + 2], F32R)
                nc.vector.tensor_copy(qaug_r[:, 0:H], q_sb[:, :])
                nc.vector.tensor_copy(qaug_r[:, H : H + 1], ones_col[:, :])
                nc.vector.tensor_copy(qaug_r[:, H + 1 : H + 2], ones_col[:, :])

                # ctxT[h, c] (f32r) and ctx (f32r) copies
                ctxT_r = ctxTp.tile([128, 2, C], F32R)
                for ht in range(2):
                    for ct in range(CT):
                        tp = ps_tp.tile([128, 128], F32, tag="tp")
                        nc.tensor.transpose(
                            tp[:, :],
                            ctx_sb[:, ct, ht * 128 : (ht + 1) * 128],
                            ident[:, :],
                        )
                        nc.scalar.copy(
                            ctxT_r[:, ht, ct * 128 : (ct + 1) * 128], tp[:, :]
                        )
                ctx_r = ctxp.tile([128, CT, H], F32R)
                for ct in range(CT):
                    nc.gpsimd.tensor_copy(ctx_r[:, ct, :], ctx_sb[:, ct, :])

                # simT[q, c] = qTs^T @ ctxT  (fp32r, N=512)
                simT = ps_sim.tile([128, C], F32)
                for ch in range(2):
                    for ht in range(2):
                        nc.tensor.matmul(
                            simT[:, ch * 512 : (ch + 1) * 512],
                            lhsT=qTs_r[:, ht, :],
                            rhs=ctxT_r[:, ht, ch * 512 : (ch + 1) * 512],
                            start=(ht == 0),
                            stop=(ht == 1),
                        )
                # expsimT = exp(simT + qvec) -> f32r, lhsT of the a-matmul
                es_r = esp.tile([128, C], F32R)
                for ch in range(2):
                    nc.scalar.activation(
                        out=es_r[:, ch * 512 : (ch + 1) * 512],
                        in_=simT[:, ch * 512 : (ch + 1) * 512],
                        func=EXP,
                        bias=qvec_col[:, 0:1],
                        scale=1.0,
                    )

                # cvec columns: cvc8[:, ct] = ctxT[:,:,ct]^T @ w_c
                cvc8 = ps_sm.tile([128, CT], F32, tag="sm")
                for ct in range(CT):
                    for ht in range(2):
                        nc.tensor.matmul(
                            cvc8[:, ct : ct + 1],
                            lhsT=ctxT_r[:, ht, ct * 128 : (ct + 1) * 128].bitcast(F32),
                            rhs=wcols[:, ht : ht + 1],
                            start=(ht == 0),
                            stop=(ht == 1),
                        )
                ecv8 = smallp.tile([128, CT], F32)
                nc.scalar.activation(
                    out=ecv8[:, :], in_=cvc8[:, :], func=EXP, bias=0.0, scale=1.0
                )

                M8w_r = smallp.tile([128, CT], F32R)
                for ct in range(CT):
                    # a-matmul with ones column: [a_unnorm | S]
                    af = ps_a.tile([128, H + 2], F32)
                    nc.tensor.matmul(
                        af[:, :],
                        lhsT=es_r[:, ct * 128 : (ct + 1) * 128],
                        rhs=qaug_r[:, :],
                        start=True,
                        stop=True,
                    )
                    rS = smallp.tile([128, 1], F32)
                    nc.vector.reciprocal(rS[:, :], af[:, H : H + 1])
                    stag = stagp.tile([128, 2, H], F32)
                    nc.vector.tensor_scalar_mul(stag[:, 0, :], af[:, 0:H], rS[:, :])
                    nc.gpsimd.tensor_mul(
                        stag[:, 1, :], ctx_sb[:, ct, :], stag[:, 0, :]
                    )
                    r0, r1 = ct * 128, (ct + 1) * 128
                    nc.sync.dma_start(out=out_d[b, r0:r1, 0:H], in_=ctx_sb[:, ct, :])
                    nc.sync.dma_start(out=out_d[b, r0:r1, H : 3 * H], in_=stag[:, :, :])

                    # beta max: transpose expsimT tile -> [c, q], rowmax from PSUM
                    mt = ps_tp.tile([128, 128], F32R, tag="tp")
                    nc.tensor.matmul(
                        mt[:, :],
                        lhsT=es_r[:, ct * 128 : (ct + 1) * 128],
                        rhs=ident_r[:, :],
                        start=True,
                        stop=True,
                        is_transpose=True,
                    )
                    M8col = smallp.tile([128, 1], F32, tag="M8col")
                    nc.vector.reduce_max(M8col[:, :], mt[:, :].bitcast(F32), axis=X)
                    # beta weight = rowmax * exp(cvec)
                    nc.vector.tensor_mul(
                        M8w_r[:, ct : ct + 1], M8col[:, :], ecv8[:, ct : ct + 1]
                    )

                # beta normalizer and bv = sum_c beta[c] ctx[c]
                Sb = ps_sm.tile([1, CT], F32, tag="sm")
                nc.tensor.matmul(
                    Sb[:, :],
                    lhsT=ones_col[:, :],
                    rhs=M8w_r[:, :].bitcast(F32),
                    start=True,
                    stop=True,
                )
                Sb1 = smallp.tile([1, 1], F32)
                nc.vector.reduce_sum(Sb1[:, :], Sb[:, :], axis=X)
                rSb = smallp.tile([1, 1], F32)
                nc.vector.reciprocal(rSb[:, :], Sb1[:, :])
                bv_ps = ps_sm.tile([1, H], F32, tag="sm")
                for ct in range(CT):
                    nc.tensor.matmul(
                        bv_ps[:, :],
                        lhsT=M8w_r[:, ct : ct + 1],
                        rhs=ctx_r[:, ct, :],
                        start=(ct == 0),
                        stop=(ct == CT - 1),
                    )
                bv_r = smallp.tile([1, H], F32R)
                nc.vector.tensor_scalar_mul(bv_r[:, :], bv_ps[:, :], rSb[:, :])
                bb_ps = ps_bb.tile([128, H], F32)
                nc.tensor.matmul(
                    bb_ps[:, :],
                    lhsT=ones_row_r[:, :],
                    rhs=bv_r[:, :],
                    start=True,
                    stop=True,
                )
                for ct in range(CT):
                    cbv = cbvp.tile([128, H], F32)
                    nc.vector.tensor_mul(cbv[:, :], ctx_sb[:, ct, :], bb_ps[:, :])
                    nc.sync.dma_start(
                        out=out_d[b, ct * 128 : (ct + 1) * 128, 3 * H : 4 * H],
                        in_=cbv[:, :],
                    )

    split_waits(nc)
    return nc


_NC = None
LAST_RESULT = None


def kernel(_trace=False, **inputs):
    global _NC, LAST_RESULT
    if _NC is None:
        _NC = build()
    context = np.ascontiguousarray(np.asarray(inputs["context"], dtype=np.float32))
    query = np.ascontiguousarray(np.asarray(inputs["query"], dtype=np.float32))
    att_w = np.ascontiguousarray(np.asarray(inputs["att_w"], dtype=np.float32))
    att_b = np.asarray(inputs["att_b"], dtype=np.float32).reshape(1)
    in_maps = [
        {
            "context": np.ascontiguousarray(context[i * BL : (i + 1) * BL]),
            "query": np.ascontiguousarray(query[i * BL : (i + 1) * BL]),
            "att_w": att_w,
            "att_b": att_b,
        }
        for i in range(NCORES)
    ]
    res = run_bass_kernel_spmd(
        _NC, in_maps, core_ids=list(range(NCORES)), trace=_trace
    )
    LAST_RESULT = res
    return np.concatenate([r["out"] for r in res.results], axis=0)


# revision 10
# speedup vs baseline: 1.2095x; 1.1073x over previous
"""BiDAF-style attention kernel for Trainium2, 8-core data-parallel over batch.

Problem (per batch b):
  sim[c,q] = ctx[c]@w_c + qry[q]@w_q + sum_h ctx[c,h] w_m[h] qry[q,h] + att_b
  alpha = softmax_q(sim);        a[c] = sum_q alpha[c,q] qry[q]
  beta  = softmax_c(max_q sim);  bv   = sum_c beta[c] ctx[c]
  out = [ctx | a | ctx*a | ctx*bv]          (C, 4H)

Key algebra:
  - ctx@w_c (cvec) is constant along q -> cancels in the alpha softmax and in
    a; it only shifts the beta logits. So sim' = sim - cvec is computed on the
    PE and cvec enters only as a tiny per-c weight exp(cvec) on the beta path.
  - att_b is a global constant -> cancels everywhere; dropped entirely.
  - No max subtraction inside softmax: logits are O(10), exp is safe in f32,
    and the shift cancels exactly.
  - max_q exp(sim') = exp(max_q sim'), so the beta max is the rowmax of the
    already-computed exp values.

Layout: sim' is built TRANSPOSED, simT [q=128 part, c=1024 free], so the main
matmuls run N=512/257 with fp32r (1 cycle/row, single pass):
  simT = qTs_r^T @ ctxT_r   (qTs = w_m * qT); qvec = qry@w_q is a
  per-partition scalar in this layout and enters via the exp bias for free.
  expsimT = exp(simT + qvec) written as f32r -> directly the lhsT of the
  a-matmul: [a | S] = expsimT^T @ [qry | 1], S = alpha normalizer from the
  ones column. The beta max comes from PE-transposing expsimT tiles back to
  [c,q] and DVE row-maxing them straight out of PSUM.
"""

import numpy as np

import concourse.bass as bass
import concourse.tile as tile
from concourse import mybir
from concourse.bass_utils import run_bass_kernel_spmd
from concourse.masks import make_identity

B, C, Q, H = 64, 1024, 128, 256
NCORES = 8
BL = B // NCORES          # batches per core
CT = C // 128             # context row-tiles per batch
F32 = mybir.dt.float32
F32R = mybir.dt.float32r


def split_waits(nc, max_waits=1):
    """walrus codegen in this container rejects >1 sem wait per instruction;
    move excess waits onto same-engine NoOps inserted just before."""
    n_new = 0
    for f in nc.m.functions:
        for blk in f.blocks:
            out = []
            for ins in blk.instructions:
                waits = list(ins.sync_info.on_wait) if ins.sync_info else []
                if len(waits) > max_waits:
                    extra, keep = waits[:-max_waits], waits[-max_waits:]
                    for j in range(0, len(extra), max_waits):
                        nop = mybir.InstNoOp(name=f"I-wsplit-{n_new}", ins=[], outs=[])
                        n_new += 1
                        nop.engine = ins.engine
                        nop.sync_info = mybir.SyncInfo(
                            on_wait=list(extra[j : j + max_waits]), on_update=[]
                        )
                        out.append(nop)
                    ins.sync_info.on_wait = list(keep)
                out.append(ins)
            blk.instructions = out
    return n_new


def build():
    nc = bass.Bass()
    ctx_d = nc.dram_tensor("context", [BL, C, H], F32, kind="ExternalInput")
    q_d = nc.dram_tensor("query", [BL, Q, H], F32, kind="ExternalInput")
    w_d = nc.dram_tensor("att_w", [3 * H], F32, kind="ExternalInput")
    b_d = nc.dram_tensor("att_b", [1], F32, kind="ExternalInput")
    out_d = nc.dram_tensor("out", [BL, C, 4 * H], F32, kind="ExternalOutput")

    X = mybir.AxisListType.X
    EXP = mybir.ActivationFunctionType.Exp

    with tile.TileContext(nc) as tc:
        from contextlib import ExitStack

        with ExitStack() as ctx:
            consts = ctx.enter_context(tc.tile_pool(name="consts", bufs=1))
            ctxp = ctx.enter_context(tc.tile_pool(name="ctx", bufs=3))
            ctxTp = ctx.enter_context(tc.tile_pool(name="ctxT", bufs=2))
            qp = ctx.enter_context(tc.tile_pool(name="qp", bufs=3))
            esp = ctx.enter_context(tc.tile_pool(name="es", bufs=2))
            stagp = ctx.enter_context(tc.tile_pool(name="stag", bufs=12))
            smallp = ctx.enter_context(tc.tile_pool(name="small", bufs=8))
            ps_sim = ctx.enter_context(tc.tile_pool(name="ps_sim", bufs=3, space="PSUM"))
            ps_tp = ctx.enter_context(tc.tile_pool(name="ps_tp", bufs=2, space="PSUM"))
            ps_a = ctx.enter_context(tc.tile_pool(name="ps_a", bufs=2, space="PSUM"))
            ps_sm = ctx.enter_context(tc.tile_pool(name="ps_sm", bufs=1, space="PSUM"))

            ident = consts.tile([128, 128], F32)
            make_identity(nc, ident[:, :])
            ident_r = consts.tile([128, 128], F32R)
            nc.vector.tensor_copy(ident_r[:, :], ident[:, :])
            ones_col = consts.tile([128, 1], F32)
            nc.vector.memset(ones_col[:, :], 1.0)
            ones_row = consts.tile([1, 128], F32)
            nc.vector.memset(ones_row[:, :], 1.0)
            ones_row_r = consts.tile([1, 128], F32R)
            nc.vector.tensor_copy(ones_row_r[:, :], ones_row[:, :])
            # att_w as 6 columns: [w_c h0|h1, w_q h0|h1, w_m h0|h1]
            wcols = consts.tile([128, 6], F32)
            nc.gpsimd.dma_start(
                out=wcols[:, :],
                in_=bass.AP(tensor=w_d, offset=0, ap=[[1, 128], [128, 6]]),
            )
            # w_q broadcast across partitions for the qvec row-reduction
            wqb = consts.tile([128, H], F32)
            nc.gpsimd.dma_start(
                out=wqb[:, :],
                in_=bass.AP(tensor=w_d, offset=H, ap=[[0, 128], [1, H]]),
            )

            for b in range(BL):
                ctx_sb = ctxp.tile([128, CT, H], F32)
                nc.scalar.dma_start(
                    out=ctx_sb[:, :, :],
                    in_=ctx_d[b].rearrange("(ct p) h -> p ct h", p=128),
                )
                q_sb = qp.tile([128, H], F32)
                nc.scalar.dma_start(out=q_sb[:, :], in_=q_d[b])

                # qT scaled by w_m -> lhsT of the simT matmul (f32r)
                qTs_r = qp.tile([128, 2, 128], F32R)
                for ht in range(2):
                    tp = ps_tp.tile([128, 128], F32, tag="tp")
                    nc.tensor.transpose(
                        tp[:, :], q_sb[:, ht * 128 : (ht + 1) * 128], ident[:, :]
                    )
                    nc.vector.tensor_scalar_mul(
                        qTs_r[:, ht, :], tp[:, :], wcols[:, 4 + ht : 5 + ht]
                    )

                # qvec[q] = qry[q] @ w_q as a column (q = partition dim)
                scr = qp.tile([128, H], F32)
                qvec_col = smallp.tile([128, 1], F32)
                nc.vector.tensor_mul(scr[:, :], q_sb[:, :], wqb[:, :])
                nc.vector.reduce_sum(qvec_col[:, :], scr[:, :], axis=X)

                # rhs of the a-matmul: [qry | 1] rounded to f32r
                qaug_r = qp.tile([128, H + 2], F32R)
                nc.vector.tensor_copy(qaug_r[:, 0:H], q_sb[:, :])
                nc.vector.tensor_copy(qaug_r[:, H : H + 1], ones_col[:, :])
                nc.vector.tensor_copy(qaug_r[:, H + 1 : H + 2], ones_col[:, :])

                # ctxT[h, c] (f32r) and ctx (f32r) copies
                ctxT_r = ctxTp.tile([128, 2, C], F32R)
                for ht in range(2):
                    for ct in range(CT):
                        tp = ps_tp.tile([128, 128], F32, tag="tp")
                        nc.tensor.transpose(
                            tp[:, :],
                            ctx_sb[:, ct, ht * 128 : (ht + 1) * 128],
                            ident[:, :],
                        )
                        nc.scalar.copy(
                            ctxT_r[:, ht, ct * 128 : (ct + 1) * 128], tp[:, :]
                        )
                # simT[q, c] = qTs^T @ ctxT  (fp32r, N=512)
                es_r = esp.tile([128, C], F32R)
                simTs = []
                for _ch in range(2):
                    simT_t = ps_sim.tile([128, 512], F32, tag="sim")
                    simTs.append(simT_t)
                for ht in range(2):
                    for ch in range(2):
                        nc.tensor.matmul(
                            simTs[ch][:, :],
                            lhsT=qTs_r[:, ht, :],
                            rhs=ctxT_r[:, ht, ch * 512 : (ch + 1) * 512],
                            start=(ht == 0),
                            stop=(ht == 1),
                        )
                for ch in range(2):
                    nc.scalar.activation(
                        out=es_r[:, ch * 512 : (ch + 1) * 512],
                        in_=simTs[ch][:, :],
                        func=EXP,
                        bias=qvec_col[:, 0:1],
                        scale=1.0,
                    )

                # cvec columns: cvc8[:, ct] = ctxT[:,:,ct]^T @ w_c
                cvc8 = ps_sm.tile([128, CT], F32, tag="sm")
                for ct in range(CT):
                    for ht in range(2):
                        nc.tensor.matmul(
                            cvc8[:, ct : ct + 1],
                            lhsT=ctxT_r[:, ht, ct * 128 : (ct + 1) * 128].bitcast(F32),
                            rhs=wcols[:, ht : ht + 1],
                            start=(ht == 0),
                            stop=(ht == 1),
                        )
                ecv8 = smallp.tile([128, CT], F32)
                nc.scalar.activation(
                    out=ecv8[:, :], in_=cvc8[:, :], func=EXP, bias=0.0, scale=1.0
                )

                M8w_r = smallp.tile([128, CT], F32R)
                stags = []
                for ct in range(CT):
                    # a-matmul with ones column: [a_unnorm | S]
                    af = ps_a.tile([128, H + 2], F32, tag="a")
                    nc.tensor.matmul(
                        af[:, :],
                        lhsT=es_r[:, ct * 128 : (ct + 1) * 128],
                        rhs=qaug_r[:, :],
                        start=True,
                        stop=True,
                    )
                    rS = smallp.tile([128, 1], F32)
                    nc.vector.reciprocal(rS[:, :], af[:, H : H + 1])
                    stag = stagp.tile([128, 4, H], F32)
                    stags.append(stag)
                    nc.gpsimd.tensor_copy(stag[:, 0, :], ctx_sb[:, ct, :])
                    nc.vector.tensor_scalar_mul(stag[:, 1, :], af[:, 0:H], rS[:, :])
                    nc.vector.tensor_mul(
                        stag[:, 2, :], ctx_sb[:, ct, :], stag[:, 1, :]
                    )

                    # beta max: transpose expsimT tile -> [c, q], rowmax from PSUM
                    mt = ps_tp.tile([128, 128], F32R, tag="tp")
                    nc.tensor.matmul(
                        mt[:, :],
                        lhsT=es_r[:, ct * 128 : (ct + 1) * 128],
                        rhs=ident_r[:, :],
                        start=True,
                        stop=True,
                        is_transpose=True,
                    )
                    M8col = smallp.tile([128, 1], F32, tag="M8col")
                    nc.vector.reduce_max(M8col[:, :], mt[:, :].bitcast(F32), axis=X)
                    # beta weight = rowmax * exp(cvec)
                    nc.vector.tensor_mul(
                        M8w_r[:, ct : ct + 1], M8col[:, :], ecv8[:, ct : ct + 1]
                    )

                # beta normalizer and bv = sum_c beta[c] ctx[c]
                Sb = ps_sm.tile([1, CT], F32, tag="sm")
                nc.tensor.matmul(
                    Sb[:, :],
                    lhsT=ones_col[:, :],
                    rhs=M8w_r[:, :].bitcast(F32),
                    start=True,
                    stop=True,
                )
                Sb1 = smallp.tile([1, 1], F32)
                nc.vector.reduce_sum(Sb1[:, :], Sb[:, :], axis=X)
                rSb = smallp.tile([1, 1], F32)
                nc.vector.reciprocal(rSb[:, :], Sb1[:, :])
                bv_ps = ps_sm.tile([1, H], F32, tag="sm")
                for ct in range(CT):
                    nc.tensor.matmul(
                        bv_ps[:, :],
                        lhsT=M8w_r[:, ct : ct + 1].bitcast(F32),
                        rhs=ctx_sb[:, ct, :],
                        start=(ct == 0),
                        stop=(ct == CT - 1),
                    )
                bv_r = smallp.tile([1, H], F32R)
                nc.vector.tensor_scalar_mul(bv_r[:, :], bv_ps[:, :], rSb[:, :])
                bb_ps = ps_a.tile([128, H + 2], F32, tag="a")
                nc.tensor.matmul(
                    bb_ps[:, 0:H],
                    lhsT=ones_row_r[:, :],
                    rhs=bv_r[:, :],
                    start=True,
                    stop=True,
                )
                for ct in range(CT):
                    nc.vector.tensor_mul(
                        stags[ct][:, 3, :], ctx_sb[:, ct, :], bb_ps[:, 0:H]
                    )
                    nc.sync.dma_start(
                        out=out_d[b, ct * 128 : (ct + 1) * 128, :],
                        in_=stags[ct][:, :, :],
                    )

    split_waits(nc)
    return nc


_NC = None
LAST_RESULT = None


def kernel(_trace=False, **inputs):
    global _NC, LAST_RESULT
    if _NC is None:
        _NC = build()
    context = np.ascontiguousarray(np.asarray(inputs["context"], dtype=np.float32))
    query = np.ascontiguousarray(np.asarray(inputs["query"], dtype=np.float32))
    att_w = np.ascontiguousarray(np.asarray(inputs["att_w"], dtype=np.float32))
    att_b = np.asarray(inputs["att_b"], dtype=np.float32).reshape(1)
    in_maps = [
        {
            "context": np.ascontiguousarray(context[i * BL : (i + 1) * BL]),
            "query": np.ascontiguousarray(query[i * BL : (i + 1) * BL]),
            "att_w": att_w,
            "att_b": att_b,
        }
        for i in range(NCORES)
    ]
    res = run_bass_kernel_spmd(
        _NC, in_maps, core_ids=list(range(NCORES)), trace=_trace
    )
    LAST_RESULT = res
    return np.concatenate([r["out"] for r in res.results], axis=0)


# revision 11
# speedup vs baseline: 1.6162x; 1.3362x over previous
"""BiDAF-style attention kernel for Trainium2, 8-core data-parallel over batch.

Problem (per batch b):
  sim[c,q] = ctx[c]@w_c + qry[q]@w_q + sum_h ctx[c,h] w_m[h] qry[q,h] + att_b
  alpha = softmax_q(sim);        a[c] = sum_q alpha[c,q] qry[q]
  beta  = softmax_c(max_q sim);  bv   = sum_c beta[c] ctx[c]
  out = [ctx | a | ctx*a | ctx*bv]          (C, 4H)

Key algebra:
  - ctx@w_c (cvec) is constant along q -> cancels in the alpha softmax and in
    a; it only shifts the beta logits. So sim' = sim - cvec is computed on the
    PE and cvec enters only as a tiny per-c weight exp(cvec) on the beta path.
  - att_b is a global constant -> cancels everywhere; dropped entirely.
  - No max subtraction inside softmax: logits are O(10), exp is safe in f32,
    and the shift cancels exactly.
  - max_q exp(sim') = exp(max_q sim'), so the beta max is the rowmax of the
    already-computed exp values.

Layout: sim' is built TRANSPOSED, simT [q=128 part, c=1024 free], so the main
matmuls run N=512/257 with fp32r (1 cycle/row, single pass):
  simT = qTs_r^T @ ctxT_r   (qTs = w_m * qT); qvec = qry@w_q is a
  per-partition scalar in this layout and enters via the exp bias for free.
  expsimT = exp(simT + qvec) written as f32r -> directly the lhsT of the
  a-matmul: [a | S] = expsimT^T @ [qry | 1], S = alpha normalizer from the
  ones column. The beta max comes from PE-transposing expsimT tiles back to
  [c,q] and DVE row-maxing them straight out of PSUM.
"""

import numpy as np

import concourse.bass as bass
import concourse.tile as tile
from concourse import mybir
from concourse.bass_utils import run_bass_kernel_spmd
from concourse.masks import make_identity

B, C, Q, H = 64, 1024, 128, 256
NCORES = 8
BL = B // NCORES          # batches per core
CT = C // 128             # context row-tiles per batch
F32 = mybir.dt.float32
F32R = mybir.dt.float32r


def split_waits(nc, max_waits=1):
    """walrus codegen in this container rejects >1 sem wait per instruction;
    move excess waits onto same-engine NoOps inserted just before."""
    n_new = 0
    for f in nc.m.functions:
        for blk in f.blocks:
            out = []
            for ins in blk.instructions:
                waits = list(ins.sync_info.on_wait) if ins.sync_info else []
                if len(waits) > max_waits:
                    extra, keep = waits[:-max_waits], waits[-max_waits:]
                    for j in range(0, len(extra), max_waits):
                        nop = mybir.InstNoOp(name=f"I-wsplit-{n_new}", ins=[], outs=[])
                        n_new += 1
                        nop.engine = ins.engine
                        nop.sync_info = mybir.SyncInfo(
                            on_wait=list(extra[j : j + max_waits]), on_update=[]
                        )
                        out.append(nop)
                    ins.sync_info.on_wait = list(keep)
                out.append(ins)
            blk.instructions = out
    return n_new


def build():
    nc = bass.Bass()
    ctx_d = nc.dram_tensor("context", [BL, C, H], F32, kind="ExternalInput")
    q_d = nc.dram_tensor("query", [BL, Q, H], F32, kind="ExternalInput")
    w_d = nc.dram_tensor("att_w", [3 * H], F32, kind="ExternalInput")
    b_d = nc.dram_tensor("att_b", [1], F32, kind="ExternalInput")
    out_d = nc.dram_tensor("out", [BL, C, 4 * H], F32, kind="ExternalOutput")

    X = mybir.AxisListType.X
    EXP = mybir.ActivationFunctionType.Exp

    with tile.TileContext(nc) as tc:
        from contextlib import ExitStack

        with ExitStack() as ctx:
            consts = ctx.enter_context(tc.tile_pool(name="consts", bufs=1))
            ctxp = ctx.enter_context(tc.tile_pool(name="ctx", bufs=3))
            ctxTp = ctx.enter_context(tc.tile_pool(name="ctxT", bufs=2))
            qp = ctx.enter_context(tc.tile_pool(name="qp", bufs=3))
            esp = ctx.enter_context(tc.tile_pool(name="es", bufs=2))
            stagp = ctx.enter_context(tc.tile_pool(name="stag", bufs=2))
            cbvp = ctx.enter_context(tc.tile_pool(name="cbv", bufs=2))
            smallp = ctx.enter_context(tc.tile_pool(name="small", bufs=8))
            ps_sim = ctx.enter_context(tc.tile_pool(name="ps_sim", bufs=2, space="PSUM"))
            ps_tp = ctx.enter_context(tc.tile_pool(name="ps_tp", bufs=2, space="PSUM"))
            ps_a = ctx.enter_context(tc.tile_pool(name="ps_a", bufs=2, space="PSUM"))
            ps_sm = ctx.enter_context(tc.tile_pool(name="ps_sm", bufs=2, space="PSUM"))

            ident = consts.tile([128, 128], F32)
            make_identity(nc, ident[:, :])
            ident_r = consts.tile([128, 128], F32R)
            nc.vector.tensor_copy(ident_r[:, :], ident[:, :])
            ones_col = consts.tile([128, 1], F32)
            nc.vector.memset(ones_col[:, :], 1.0)
            ones_row = consts.tile([1, 128], F32)
            nc.vector.memset(ones_row[:, :], 1.0)
            ones_row_r = consts.tile([1, 128], F32R)
            nc.vector.tensor_copy(ones_row_r[:, :], ones_row[:, :])
            # att_w as 6 columns: [w_c h0|h1, w_q h0|h1, w_m h0|h1]
            wcols = consts.tile([128, 6], F32)
            nc.gpsimd.dma_start(
                out=wcols[:, :],
                in_=bass.AP(tensor=w_d, offset=0, ap=[[1, 128], [128, 6]]),
            )
            wc_r = consts.tile([128, 2], F32R)
            nc.vector.tensor_copy(wc_r[:, :], wcols[:, 0:2])
            # w_q broadcast across partitions for the qvec row-reduction
            wqb = consts.tile([128, H], F32)
            nc.gpsimd.dma_start(
                out=wqb[:, :],
                in_=bass.AP(tensor=w_d, offset=H, ap=[[0, 128], [1, H]]),
            )

            for b in range(BL):
                ctx_sb = ctxp.tile([128, CT, H], F32)
                nc.scalar.dma_start(
                    out=ctx_sb[:, :, :],
                    in_=ctx_d[b].rearrange("(ct p) h -> p ct h", p=128),
                )
                q_sb = qp.tile([128, H], F32)
                nc.scalar.dma_start(out=q_sb[:, :], in_=q_d[b])
                nc.sync.dma_start(
                    out=out_d[b, :, 0:H].rearrange("(ct p) h -> p ct h", p=128),
                    in_=ctx_sb[:, :, :],
                )

                # qT scaled by w_m -> lhsT of the simT matmul (f32r)
                qTs_r = qp.tile([128, 2, 128], F32R)
                for ht in range(2):
                    tp = ps_tp.tile([128, 128], F32, tag="tp")
                    nc.tensor.transpose(
                        tp[:, :], q_sb[:, ht * 128 : (ht + 1) * 128], ident[:, :]
                    )
                    nc.vector.tensor_scalar_mul(
                        qTs_r[:, ht, :], tp[:, :], wcols[:, 4 + ht : 5 + ht]
                    )

                # qvec[q] = qry[q] @ w_q as a column (q = partition dim)
                scr = qp.tile([128, H], F32)
                qvec_col = smallp.tile([128, 1], F32)
                nc.vector.tensor_mul(scr[:, :], q_sb[:, :], wqb[:, :])
                nc.vector.reduce_sum(qvec_col[:, :], scr[:, :], axis=X)

                # rhs of the a-matmul: [qry | 1] rounded to f32r
                qaug_r = qp.tile([128, H + 2], F32R)
                nc.vector.tensor_copy(qaug_r[:, 0:H], q_sb[:, :])
                nc.vector.tensor_copy(qaug_r[:, H : H + 1], ones_col[:, :])
                nc.vector.tensor_copy(qaug_r[:, H + 1 : H + 2], ones_col[:, :])

                # ctxT[h, c] (f32r) and ctx (f32r) copies
                ctxT_r = ctxTp.tile([128, 2, C], F32R)
                for ht in range(2):
                    for ct in range(CT):
                        tp = ps_tp.tile([128, 128], F32, tag="tp")
                        nc.tensor.transpose(
                            tp[:, :],
                            ctx_sb[:, ct, ht * 128 : (ht + 1) * 128],
                            ident[:, :],
                        )
                        nc.scalar.copy(
                            ctxT_r[:, ht, ct * 128 : (ct + 1) * 128], tp[:, :]
                        )
                ctx_r = ctxp.tile([128, CT, H], F32R)
                nc.vector.tensor_copy(ctx_r[:, :, :], ctx_sb[:, :, :])

                # simT[q, c] = qTs^T @ ctxT  (fp32r, N=512)
                es_r = esp.tile([128, C], F32R)
                simTs = []
                for _ch in range(2):
                    simT_t = ps_sim.tile([128, 512], F32, tag="sim")
                    simTs.append(simT_t)
                for ht in range(2):
                    for ch in range(2):
                        nc.tensor.matmul(
                            simTs[ch][:, :],
                            lhsT=qTs_r[:, ht, :],
                            rhs=ctxT_r[:, ht, ch * 512 : (ch + 1) * 512],
                            start=(ht == 0),
                            stop=(ht == 1),
                        )
                for ch in range(2):
                    nc.scalar.activation(
                        out=es_r[:, ch * 512 : (ch + 1) * 512],
                        in_=simTs[ch][:, :],
                        func=EXP,
                        bias=qvec_col[:, 0:1],
                        scale=1.0,
                    )

                # cvec row: w_c^T @ ctxT (fp32r, N=512), then row -> columns
                cvec_sb = smallp.tile([1, C], F32, tag="cvec")
                for ch in range(2):
                    cvr = ps_sm.tile([1, 512], F32, tag="sm")
                    for ht in range(2):
                        nc.tensor.matmul(
                            cvr[:, :],
                            lhsT=wc_r[:, ht : ht + 1],
                            rhs=ctxT_r[:, ht, ch * 512 : (ch + 1) * 512],
                            start=(ht == 0),
                            stop=(ht == 1),
                        )
                    nc.scalar.copy(cvec_sb[:, ch * 512 : (ch + 1) * 512], cvr[:, :])
                ecv_ps = ps_sm.tile([128, CT], F32, tag="sm")
                for ct in range(CT):
                    nc.tensor.transpose(
                        ecv_ps[:, ct : ct + 1],
                        cvec_sb[0:1, ct * 128 : (ct + 1) * 128],
                        ident[0:1, 0:1],
                    )
                ecv8 = smallp.tile([128, CT], F32)
                nc.scalar.activation(
                    out=ecv8[:, :], in_=ecv_ps[:, :], func=EXP, bias=0.0, scale=1.0
                )

                M8w_r = smallp.tile([128, CT], F32R)
                actxa = stagp.tile([128, CT, 2, H], F32)
                cbv8 = cbvp.tile([128, CT, H], F32)
                for ct in range(CT):
                    # a-matmul with ones column: [a_unnorm | S]
                    af = ps_a.tile([128, H + 2], F32, tag="a")
                    nc.tensor.matmul(
                        af[:, :],
                        lhsT=es_r[:, ct * 128 : (ct + 1) * 128],
                        rhs=qaug_r[:, :],
                        start=True,
                        stop=True,
                    )
                    rS = smallp.tile([128, 1], F32)
                    nc.vector.reciprocal(rS[:, :], af[:, H : H + 1])
                    nc.vector.tensor_scalar_mul(actxa[:, ct, 0, :], af[:, 0:H], rS[:, :])
                    nc.gpsimd.tensor_mul(
                        actxa[:, ct, 1, :], ctx_sb[:, ct, :], actxa[:, ct, 0, :]
                    )

                    # beta max: transpose expsimT tile -> [c, q], rowmax from PSUM
                    mt = ps_tp.tile([128, 128], F32R, tag="tp")
                    nc.tensor.matmul(
                        mt[:, :],
                        lhsT=es_r[:, ct * 128 : (ct + 1) * 128],
                        rhs=ident_r[:, :],
                        start=True,
                        stop=True,
                        is_transpose=True,
                    )
                    M8col = smallp.tile([128, 1], F32, tag="M8col")
                    nc.vector.reduce_max(M8col[:, :], mt[:, :].bitcast(F32), axis=X)
                    # beta weight = rowmax * exp(cvec)
                    nc.vector.tensor_mul(
                        M8w_r[:, ct : ct + 1], M8col[:, :], ecv8[:, ct : ct + 1]
                    )

                # beta normalizer and bv = sum_c beta[c] ctx[c]
                Sb = ps_sm.tile([1, CT], F32, tag="sm")
                nc.tensor.matmul(
                    Sb[:, :],
                    lhsT=ones_col[:, :],
                    rhs=M8w_r[:, :].bitcast(F32),
                    start=True,
                    stop=True,
                )
                Sb1 = smallp.tile([1, 1], F32)
                nc.vector.reduce_sum(Sb1[:, :], Sb[:, :], axis=X)
                rSb = smallp.tile([1, 1], F32)
                nc.vector.reciprocal(rSb[:, :], Sb1[:, :])
                bv_ps = ps_sm.tile([1, H], F32, tag="sm")
                for ct in range(CT):
                    nc.tensor.matmul(
                        bv_ps[:, :],
                        lhsT=M8w_r[:, ct : ct + 1],
                        rhs=ctx_r[:, ct, :],
                        start=(ct == 0),
                        stop=(ct == CT - 1),
                    )
                bv_r = smallp.tile([1, H], F32R)
                nc.vector.tensor_scalar_mul(bv_r[:, :], bv_ps[:, :], rSb[:, :])
                bb_ps = ps_a.tile([128, H + 2], F32, tag="a")
                nc.tensor.matmul(
                    bb_ps[:, 0:H],
                    lhsT=ones_row_r[:, :],
                    rhs=bv_r[:, :],
                    start=True,
                    stop=True,
                )
                for ct in range(CT):
                    nc.vector.tensor_mul(
                        cbv8[:, ct, :], ctx_sb[:, ct, :], bb_ps[:, 0:H]
                    )
                nc.sync.dma_start(
                    out=out_d[b, :, H : 3 * H].rearrange("(ct p) h -> p ct h", p=128),
                    in_=actxa[:, :, :, :],
                )
                nc.sync.dma_start(
                    out=out_d[b, :, 3 * H : 4 * H].rearrange("(ct p) h -> p ct h", p=128),
                    in_=cbv8[:, :, :],
                )

    split_waits(nc)
    return nc


_NC = None
LAST_RESULT = None


def kernel(_trace=False, **inputs):
    global _NC, LAST_RESULT
    if _NC is None:
        _NC = build()
    context = np.ascontiguousarray(np.asarray(inputs["context"], dtype=np.float32))
    query = np.ascontiguousarray(np.asarray(inputs["query"], dtype=np.float32))
    att_w = np.ascontiguousarray(np.asarray(inputs["att_w"], dtype=np.float32))
    att_b = np.asarray(inputs["att_b"], dtype=np.float32).reshape(1)
    in_maps = [
        {
            "context": np.ascontiguousarray(context[i * BL : (i + 1) * BL]),
            "query": np.ascontiguousarray(query[i * BL : (i + 1) * BL]),
            "att_w": att_w,
            "att_b": att_b,
        }
        for i in range(NCORES)
    ]
    res = run_bass_kernel_spmd(
        _NC, in_maps, core_ids=list(range(NCORES)), trace=_trace
    )
    LAST_RESULT = res
    return np.concatenate([r["out"] for r in res.results], axis=0)


# revision 12
# speedup vs baseline: 1.6344x; 1.0113x over previous
"""BiDAF-style attention kernel for Trainium2, 8-core data-parallel over batch.

Problem (per batch b):
  sim[c,q] = ctx[c]@w_c + qry[q]@w_q + sum_h ctx[c,h] w_m[h] qry[q,h] + att_b
  alpha = softmax_q(sim);        a[c] = sum_q alpha[c,q] qry[q]
  beta  = softmax_c(max_q sim);  bv   = sum_c beta[c] ctx[c]
  out = [ctx | a | ctx*a | ctx*bv]          (C, 4H)

Key algebra:
  - ctx@w_c (cvec) is constant along q -> cancels in the alpha softmax and in
    a; it only shifts the beta logits. So sim' = sim - cvec is computed on the
    PE and cvec enters only as a tiny per-c weight exp(cvec) on the beta path.
  - att_b is a global constant -> cancels everywhere; dropped entirely.
  - No max subtraction inside softmax: logits are O(10), exp is safe in f32,
    and the shift cancels exactly.
  - max_q exp(sim') = exp(max_q sim'), so the beta max is the rowmax of the
    already-computed exp values.

Layout: sim' is built TRANSPOSED, simT [q=128 part, c=1024 free], so the main
matmuls run N=512/257 with fp32r (1 cycle/row, single pass):
  simT = qTs_r^T @ ctxT_r   (qTs = w_m * qT); qvec = qry@w_q is a
  per-partition scalar in this layout and enters via the exp bias for free.
  expsimT = exp(simT + qvec) written as f32r -> directly the lhsT of the
  a-matmul: [a | S] = expsimT^T @ [qry | 1], S = alpha normalizer from the
  ones column. The beta max comes from PE-transposing expsimT tiles back to
  [c,q] and DVE row-maxing them straight out of PSUM.
"""

import numpy as np

import concourse.bass as bass
import concourse.tile as tile
from concourse import mybir
from concourse.bass_utils import run_bass_kernel_spmd
from concourse.masks import make_identity

B, C, Q, H = 64, 1024, 128, 256
NCORES = 8
BL = B // NCORES          # batches per core
CT = C // 128             # context row-tiles per batch
F32 = mybir.dt.float32
F32R = mybir.dt.float32r


def split_waits(nc, max_waits=1):
    """walrus codegen in this container rejects >1 sem wait per instruction;
    move excess waits onto same-engine NoOps inserted just before."""
    n_new = 0
    for f in nc.m.functions:
        for blk in f.blocks:
            out = []
            for ins in blk.instructions:
                waits = list(ins.sync_info.on_wait) if ins.sync_info else []
                if len(waits) > max_waits:
                    extra, keep = waits[:-max_waits], waits[-max_waits:]
                    for j in range(0, len(extra), max_waits):
                        nop = mybir.InstNoOp(name=f"I-wsplit-{n_new}", ins=[], outs=[])
                        n_new += 1
                        nop.engine = ins.engine
                        nop.sync_info = mybir.SyncInfo(
                            on_wait=list(extra[j : j + max_waits]), on_update=[]
                        )
                        out.append(nop)
                    ins.sync_info.on_wait = list(keep)
                out.append(ins)
            blk.instructions = out
    return n_new


def build():
    nc = bass.Bass()
    ctx_d = nc.dram_tensor("context", [BL, C, H], F32, kind="ExternalInput")
    q_d = nc.dram_tensor("query", [BL, Q, H], F32, kind="ExternalInput")
    w_d = nc.dram_tensor("att_w", [3 * H], F32, kind="ExternalInput")
    b_d = nc.dram_tensor("att_b", [1], F32, kind="ExternalInput")
    out_d = nc.dram_tensor("out", [BL, C, 4 * H], F32, kind="ExternalOutput")

    X = mybir.AxisListType.X
    EXP = mybir.ActivationFunctionType.Exp

    with tile.TileContext(nc) as tc:
        from contextlib import ExitStack

        with ExitStack() as ctx:
            consts = ctx.enter_context(tc.tile_pool(name="consts", bufs=1))
            ctxp = ctx.enter_context(tc.tile_pool(name="ctx", bufs=3))
            ctxTp = ctx.enter_context(tc.tile_pool(name="ctxT", bufs=3))
            qp = ctx.enter_context(tc.tile_pool(name="qp", bufs=3))
            esp = ctx.enter_context(tc.tile_pool(name="es", bufs=3))
            stagp = ctx.enter_context(tc.tile_pool(name="stag", bufs=2))
            cbvp = ctx.enter_context(tc.tile_pool(name="cbv", bufs=2))
            smallp = ctx.enter_context(tc.tile_pool(name="small", bufs=8))
            ps_sim = ctx.enter_context(tc.tile_pool(name="ps_sim", bufs=2, space="PSUM"))
            ps_tp = ctx.enter_context(tc.tile_pool(name="ps_tp", bufs=2, space="PSUM"))
            ps_a = ctx.enter_context(tc.tile_pool(name="ps_a", bufs=2, space="PSUM"))
            ps_sm = ctx.enter_context(tc.tile_pool(name="ps_sm", bufs=2, space="PSUM"))

            ident = consts.tile([128, 128], F32)
            make_identity(nc, ident[:, :])
            ident_r = consts.tile([128, 128], F32R)
            nc.vector.tensor_copy(ident_r[:, :], ident[:, :])
            ones_col = consts.tile([128, 1], F32)
            nc.vector.memset(ones_col[:, :], 1.0)
            ones_row = consts.tile([1, 128], F32)
            nc.vector.memset(ones_row[:, :], 1.0)
            ones_row_r = consts.tile([1, 128], F32R)
            nc.vector.tensor_copy(ones_row_r[:, :], ones_row[:, :])
            # att_w as 6 columns: [w_c h0|h1, w_q h0|h1, w_m h0|h1]
            wcols = consts.tile([128, 6], F32)
            nc.gpsimd.dma_start(
                out=wcols[:, :],
                in_=bass.AP(tensor=w_d, offset=0, ap=[[1, 128], [128, 6]]),
            )
            wc_r = consts.tile([128, 2], F32R)
            nc.vector.tensor_copy(wc_r[:, :], wcols[:, 0:2])
            # w_q broadcast across partitions for the qvec row-reduction
            wqb = consts.tile([128, H], F32)
            nc.gpsimd.dma_start(
                out=wqb[:, :],
                in_=bass.AP(tensor=w_d, offset=H, ap=[[0, 128], [1, H]]),
            )

            for b in range(BL):
                ctx_sb = ctxp.tile([128, CT, H], F32)
                nc.scalar.dma_start(
                    out=ctx_sb[:, :, :],
                    in_=ctx_d[b].rearrange("(ct p) h -> p ct h", p=128),
                )
                q_sb = qp.tile([128, H], F32)
                nc.scalar.dma_start(out=q_sb[:, :], in_=q_d[b])
                nc.sync.dma_start(
                    out=out_d[b, :, 0:H].rearrange("(ct p) h -> p ct h", p=128),
                    in_=ctx_sb[:, :, :],
                )

                # qT scaled by w_m -> lhsT of the simT matmul (f32r)
                qTs_r = qp.tile([128, 2, 128], F32R)
                for ht in range(2):
                    tp = ps_tp.tile([128, 128], F32, tag="tp")
                    nc.tensor.transpose(
                        tp[:, :], q_sb[:, ht * 128 : (ht + 1) * 128], ident[:, :]
                    )
                    nc.vector.tensor_scalar_mul(
                        qTs_r[:, ht, :], tp[:, :], wcols[:, 4 + ht : 5 + ht]
                    )

                # qvec[q] = qry[q] @ w_q as a column (q = partition dim)
                scr = qp.tile([128, H], F32)
                qvec_col = smallp.tile([128, 1], F32)
                nc.vector.tensor_mul(scr[:, :], q_sb[:, :], wqb[:, :])
                nc.vector.reduce_sum(qvec_col[:, :], scr[:, :], axis=X)

                # rhs of the a-matmul: [qry | 1] rounded to f32r
                qaug_r = qp.tile([128, H + 2], F32R)
                nc.vector.tensor_copy(qaug_r[:, 0:H], q_sb[:, :])
                nc.vector.tensor_copy(qaug_r[:, H : H + 1], ones_col[:, :])
                nc.vector.tensor_copy(qaug_r[:, H + 1 : H + 2], ones_col[:, :])

                # rounded ctx once; f32r transposes (1.5 cyc/row) for ctxT
                ctx_r = ctxp.tile([128, CT, H], F32R)
                nc.vector.tensor_copy(ctx_r[:, :, :], ctx_sb[:, :, :])
                ctxT_r = ctxTp.tile([128, 2, C], F32R)
                for ht in range(2):
                    for ct in range(CT):
                        tp = ps_tp.tile([128, 128], F32R, tag="tp")
                        nc.tensor.matmul(
                            tp[:, :],
                            lhsT=ctx_r[:, ct, ht * 128 : (ht + 1) * 128],
                            rhs=ident_r[:, :],
                            start=True,
                            stop=True,
                            is_transpose=True,
                        )
                        nc.scalar.copy(
                            ctxT_r[:, ht, ct * 128 : (ct + 1) * 128], tp[:, :].bitcast(F32)
                        )

                # simT[q, c] = qTs^T @ ctxT  (fp32r, N=512)
                es_r = esp.tile([128, C], F32R)
                simTs = []
                for _ch in range(2):
                    simT_t = ps_sim.tile([128, 512], F32, tag="sim")
                    simTs.append(simT_t)
                for ht in range(2):
                    for ch in range(2):
                        nc.tensor.matmul(
                            simTs[ch][:, :],
                            lhsT=qTs_r[:, ht, :],
                            rhs=ctxT_r[:, ht, ch * 512 : (ch + 1) * 512],
                            start=(ht == 0),
                            stop=(ht == 1),
                        )
                for ch in range(2):
                    nc.scalar.activation(
                        out=es_r[:, ch * 512 : (ch + 1) * 512],
                        in_=simTs[ch][:, :],
                        func=EXP,
                        bias=qvec_col[:, 0:1],
                        scale=1.0,
                    )

                # cvec row: w_c^T @ ctxT (fp32r, N=512), then row -> columns
                cvec_sb = smallp.tile([1, C], F32, tag="cvec")
                for ch in range(2):
                    cvr = ps_sm.tile([1, 512], F32, tag="sm")
                    for ht in range(2):
                        nc.tensor.matmul(
                            cvr[:, :],
                            lhsT=wc_r[:, ht : ht + 1],
                            rhs=ctxT_r[:, ht, ch * 512 : (ch + 1) * 512],
                            start=(ht == 0),
                            stop=(ht == 1),
                        )
                    nc.scalar.copy(cvec_sb[:, ch * 512 : (ch + 1) * 512], cvr[:, :])
                ecv_ps = ps_sm.tile([128, CT], F32, tag="sm")
                for ct in range(CT):
                    nc.tensor.transpose(
                        ecv_ps[:, ct : ct + 1],
                        cvec_sb[0:1, ct * 128 : (ct + 1) * 128],
                        ident[0:1, 0:1],
                    )
                ecv8 = smallp.tile([128, CT], F32)
                nc.scalar.activation(
                    out=ecv8[:, :], in_=ecv_ps[:, :], func=EXP, bias=0.0, scale=1.0
                )

                M8 = smallp.tile([128, CT], F32)
                actxa = stagp.tile([128, CT, 2, H], F32)
                cbv8 = cbvp.tile([128, CT, H], F32)
                for ct in range(CT):
                    # a-matmul with ones column: [a_unnorm | S]
                    af = ps_a.tile([128, H + 2], F32, tag="a")
                    nc.tensor.matmul(
                        af[:, :],
                        lhsT=es_r[:, ct * 128 : (ct + 1) * 128],
                        rhs=qaug_r[:, :],
                        start=True,
                        stop=True,
                    )
                    rS = smallp.tile([128, 1], F32)
                    nc.vector.reciprocal(rS[:, :], af[:, H : H + 1])
                    nc.vector.tensor_scalar_mul(actxa[:, ct, 0, :], af[:, 0:H], rS[:, :])
                    nc.gpsimd.tensor_mul(
                        actxa[:, ct, 1, :], ctx_sb[:, ct, :], actxa[:, ct, 0, :]
                    )

                    # beta max: transpose expsimT tile -> [c, q], rowmax from PSUM
                    mt = ps_tp.tile([128, 128], F32R, tag="tp")
                    nc.tensor.matmul(
                        mt[:, :],
                        lhsT=es_r[:, ct * 128 : (ct + 1) * 128],
                        rhs=ident_r[:, :],
                        start=True,
                        stop=True,
                        is_transpose=True,
                    )
                    nc.vector.reduce_max(
                        M8[:, ct : ct + 1], mt[:, :].bitcast(F32), axis=X
                    )

                # beta weights = rowmax * exp(cvec), then normalizer and bv
                M8w_r = smallp.tile([128, CT], F32R)
                nc.vector.tensor_mul(M8w_r[:, :], M8[:, :], ecv8[:, :])
                Sb = ps_sm.tile([1, CT], F32, tag="sm")
                nc.tensor.matmul(
                    Sb[:, :],
                    lhsT=ones_col[:, :],
                    rhs=M8w_r[:, :].bitcast(F32),
                    start=True,
                    stop=True,
                )
                Sb1 = smallp.tile([1, 1], F32)
                nc.vector.reduce_sum(Sb1[:, :], Sb[:, :], axis=X)
                rSb = smallp.tile([1, 1], F32)
                nc.vector.reciprocal(rSb[:, :], Sb1[:, :])
                bv_ps = ps_sm.tile([1, H], F32, tag="sm")
                for ct in range(CT):
                    nc.tensor.matmul(
                        bv_ps[:, :],
                        lhsT=M8w_r[:, ct : ct + 1],
                        rhs=ctx_r[:, ct, :],
                        start=(ct == 0),
                        stop=(ct == CT - 1),
                    )
                bv_r = smallp.tile([1, H], F32R)
                nc.vector.tensor_scalar_mul(bv_r[:, :], bv_ps[:, :], rSb[:, :])
                bb_ps = ps_a.tile([128, H + 2], F32, tag="a")
                nc.tensor.matmul(
                    bb_ps[:, 0:H],
                    lhsT=ones_row_r[:, :],
                    rhs=bv_r[:, :],
                    start=True,
                    stop=True,
                )
                bb_bcast = bass.AP(
                    tensor=bb_ps.tensor,
                    offset=bb_ps[:, 0:H].offset,
                    ap=[bb_ps[:, 0:H].ap[0], [0, CT], [1, H]],
                )
                nc.vector.tensor_mul(cbv8[:, :, :], ctx_sb[:, :, :], bb_bcast)
                nc.sync.dma_start(
                    out=out_d[b, :, H : 3 * H].rearrange("(ct p) h -> p ct h", p=128),
                    in_=actxa[:, :, :, :],
                )
                nc.sync.dma_start(
                    out=out_d[b, :, 3 * H : 4 * H].rearrange("(ct p) h -> p ct h", p=128),
                    in_=cbv8[:, :, :],
                )

    split_waits(nc)
    return nc


_NC = None
LAST_RESULT = None


def kernel(_trace=False, **inputs):
    global _NC, LAST_RESULT
    if _NC is None:
        _NC = build()
    context = np.ascontiguousarray(np.asarray(inputs["context"], dtype=np.float32))
    query = np.ascontiguousarray(np.asarray(inputs["query"], dtype=np.float32))
    att_w = np.ascontiguousarray(np.asarray(inputs["att_w"], dtype=np.float32))
    att_b = np.asarray(inputs["att_b"], dtype=np.float32).reshape(1)
    in_maps = [
        {
            "context": np.ascontiguousarray(context[i * BL : (i + 1) * BL]),
            "query": np.ascontiguousarray(query[i * BL : (i + 1) * BL]),
            "att_w": att_w,
            "att_b": att_b,
        }
        for i in range(NCORES)
    ]
    res = run_bass_kernel_spmd(
        _NC, in_maps, core_ids=list(range(NCORES)), trace=_trace
    )
    LAST_RESULT = res
    return np.concatenate([r["out"] for r in res.results], axis=0)


# revision 13
# speedup vs baseline: 1.6422x; 1.0048x over previous
"""BiDAF-style attention kernel for Trainium2, 8-core data-parallel over batch.

Problem (per batch b):
  sim[c,q] = ctx[c]@w_c + qry[q]@w_q + sum_h ctx[c,h] w_m[h] qry[q,h] + att_b
  alpha = softmax_q(sim);        a[c] = sum_q alpha[c,q] qry[q]
  beta  = softmax_c(max_q sim);  bv   = sum_c beta[c] ctx[c]
  out = [ctx | a | ctx*a | ctx*bv]          (C, 4H)

Key algebra:
  - ctx@w_c (cvec) is constant along q -> cancels in the alpha softmax and in
    a; it only shifts the beta logits. So sim' = sim - cvec is computed on the
    PE and cvec enters only as a tiny per-c weight exp(cvec) on the beta path.
  - att_b is a global constant -> cancels everywhere; dropped entirely.
  - No max subtraction inside softmax: logits are O(10), exp is safe in f32,
    and the shift cancels exactly.
  - max_q exp(sim') = exp(max_q sim'), so the beta max is the rowmax of the
    already-computed exp values.

Layout: sim' is built TRANSPOSED, simT [q=128 part, c=1024 free], so the main
matmuls run N=512/257 with fp32r (1 cycle/row, single pass):
  simT = qTs_r^T @ ctxT_r   (qTs = w_m * qT); qvec = qry@w_q is a
  per-partition scalar in this layout and enters via the exp bias for free.
  expsimT = exp(simT + qvec) written as f32r -> directly the lhsT of the
  a-matmul: [a | S] = expsimT^T @ [qry | 1], S = alpha normalizer from the
  ones column. The beta max comes from PE-transposing expsimT tiles back to
  [c,q] and DVE row-maxing them straight out of PSUM.
"""

import numpy as np

import concourse.bass as bass
import concourse.tile as tile
from concourse import mybir
from concourse.bass_utils import run_bass_kernel_spmd
from concourse.masks import make_identity

B, C, Q, H = 64, 1024, 128, 256
NCORES = 8
BL = B // NCORES          # batches per core
CT = C // 128             # context row-tiles per batch
F32 = mybir.dt.float32
F32R = mybir.dt.float32r


def split_waits(nc, max_waits=1):
    """walrus codegen in this container rejects >1 sem wait per instruction;
    move excess waits onto same-engine NoOps inserted just before."""
    n_new = 0
    for f in nc.m.functions:
        for blk in f.blocks:
            out = []
            for ins in blk.instructions:
                waits = list(ins.sync_info.on_wait) if ins.sync_info else []
                if len(waits) > max_waits:
                    extra, keep = waits[:-max_waits], waits[-max_waits:]
                    for j in range(0, len(extra), max_waits):
                        nop = mybir.InstNoOp(name=f"I-wsplit-{n_new}", ins=[], outs=[])
                        n_new += 1
                        nop.engine = ins.engine
                        nop.sync_info = mybir.SyncInfo(
                            on_wait=list(extra[j : j + max_waits]), on_update=[]
                        )
                        out.append(nop)
                    ins.sync_info.on_wait = list(keep)
                out.append(ins)
            blk.instructions = out
    return n_new


def build():
    nc = bass.Bass()
    ctx_d = nc.dram_tensor("context", [BL, C, H], F32, kind="ExternalInput")
    q_d = nc.dram_tensor("query", [BL, Q, H], F32, kind="ExternalInput")
    w_d = nc.dram_tensor("att_w", [3 * H], F32, kind="ExternalInput")
    b_d = nc.dram_tensor("att_b", [1], F32, kind="ExternalInput")
    out_d = nc.dram_tensor("out", [BL, C, 4 * H], F32, kind="ExternalOutput")

    X = mybir.AxisListType.X
    EXP = mybir.ActivationFunctionType.Exp

    with tile.TileContext(nc) as tc:
        from contextlib import ExitStack

        with ExitStack() as ctx:
            consts = ctx.enter_context(tc.tile_pool(name="consts", bufs=1))
            ctxp = ctx.enter_context(tc.tile_pool(name="ctx", bufs=3))
            ctxTp = ctx.enter_context(tc.tile_pool(name="ctxT", bufs=3))
            qp = ctx.enter_context(tc.tile_pool(name="qp", bufs=3))
            esp = ctx.enter_context(tc.tile_pool(name="es", bufs=3))
            stagp = ctx.enter_context(tc.tile_pool(name="stag", bufs=2))
            cbvp = ctx.enter_context(tc.tile_pool(name="cbv", bufs=2))
            smallp = ctx.enter_context(tc.tile_pool(name="small", bufs=8))
            ps_sim = ctx.enter_context(tc.tile_pool(name="ps_sim", bufs=2, space="PSUM"))
            ps_tp = ctx.enter_context(tc.tile_pool(name="ps_tp", bufs=3, space="PSUM"))
            ps_a = ctx.enter_context(tc.tile_pool(name="ps_a", bufs=2, space="PSUM"))
            ps_sm = ctx.enter_context(tc.tile_pool(name="ps_sm", bufs=1, space="PSUM"))

            ident = consts.tile([128, 128], F32)
            make_identity(nc, ident[:, :])
            ident_r = consts.tile([128, 128], F32R)
            nc.vector.tensor_copy(ident_r[:, :], ident[:, :])
            ones_col = consts.tile([128, 1], F32)
            nc.vector.memset(ones_col[:, :], 1.0)
            ones_row = consts.tile([1, 128], F32)
            nc.vector.memset(ones_row[:, :], 1.0)
            ones_row_r = consts.tile([1, 128], F32R)
            nc.vector.tensor_copy(ones_row_r[:, :], ones_row[:, :])
            # att_w as 6 columns: [w_c h0|h1, w_q h0|h1, w_m h0|h1]
            wcols = consts.tile([128, 6], F32)
            nc.gpsimd.dma_start(
                out=wcols[:, :],
                in_=bass.AP(tensor=w_d, offset=0, ap=[[1, 128], [128, 6]]),
            )
            wc_r = consts.tile([128, 2], F32R)
            nc.vector.tensor_copy(wc_r[:, :], wcols[:, 0:2])
            # w_q broadcast across partitions for the qvec row-reduction
            wqb = consts.tile([128, H], F32)
            nc.gpsimd.dma_start(
                out=wqb[:, :],
                in_=bass.AP(tensor=w_d, offset=H, ap=[[0, 128], [1, H]]),
            )

            for b in range(BL):
                ctx_sb = ctxp.tile([128, CT, H], F32)
                nc.scalar.dma_start(
                    out=ctx_sb[:, :, :],
                    in_=ctx_d[b].rearrange("(ct p) h -> p ct h", p=128),
                )
                q_sb = qp.tile([128, H], F32)
                nc.scalar.dma_start(out=q_sb[:, :], in_=q_d[b])
                nc.sync.dma_start(
                    out=out_d[b, :, 0:H].rearrange("(ct p) h -> p ct h", p=128),
                    in_=ctx_sb[:, :, :],
                )

                # qT scaled by w_m -> lhsT of the simT matmul (f32r)
                qTs_r = qp.tile([128, 2, 128], F32R)
                for ht in range(2):
                    tp = ps_tp.tile([128, 128], F32, tag="tp")
                    nc.tensor.transpose(
                        tp[:, :], q_sb[:, ht * 128 : (ht + 1) * 128], ident[:, :]
                    )
                    nc.vector.tensor_scalar_mul(
                        qTs_r[:, ht, :], tp[:, :], wcols[:, 4 + ht : 5 + ht]
                    )

                # qvec[q] = qry[q] @ w_q as a column (q = partition dim)
                scr = qp.tile([128, H], F32)
                qvec_col = smallp.tile([128, 1], F32)
                nc.vector.tensor_mul(scr[:, :], q_sb[:, :], wqb[:, :])
                nc.vector.reduce_sum(qvec_col[:, :], scr[:, :], axis=X)

                # rhs of the a-matmul: [qry | 1] rounded to f32r
                qaug_r = qp.tile([128, H + 2], F32R)
                nc.vector.tensor_copy(qaug_r[:, 0:H], q_sb[:, :])
                nc.vector.tensor_copy(qaug_r[:, H : H + 1], ones_col[:, :])
                nc.vector.tensor_copy(qaug_r[:, H + 1 : H + 2], ones_col[:, :])

                # rounded ctx once; f32r transposes (1.5 cyc/row) for ctxT
                ctx_r = ctxp.tile([128, CT, H], F32R)
                nc.scalar.copy(ctx_r[:, :, :], ctx_sb[:, :, :])
                ctxT_r = ctxTp.tile([128, 2, C], F32R)
                for ht in range(2):
                    for ct in range(CT):
                        tp = ps_tp.tile([128, 128], F32R, tag="tp")
                        nc.tensor.matmul(
                            tp[:, :],
                            lhsT=ctx_r[:, ct, ht * 128 : (ht + 1) * 128],
                            rhs=ident_r[:, :],
                            start=True,
                            stop=True,
                            is_transpose=True,
                        )
                        nc.scalar.copy(
                            ctxT_r[:, ht, ct * 128 : (ct + 1) * 128], tp[:, :].bitcast(F32)
                        )

                # cvec row: w_c^T @ ctxT (fp32r, N=512) -> rounded SBUF row
                cvec_r = smallp.tile([1, C], F32R, tag="cvec")
                for ch in range(2):
                    cvr = ps_sm.tile([1, 512], F32, tag="sm")
                    for ht in range(2):
                        nc.tensor.matmul(
                            cvr[:, :],
                            lhsT=wc_r[:, ht : ht + 1],
                            rhs=ctxT_r[:, ht, ch * 512 : (ch + 1) * 512],
                            start=(ht == 0),
                            stop=(ht == 1),
                        )
                    nc.scalar.copy(cvec_r[:, ch * 512 : (ch + 1) * 512], cvr[:, :])

                # simT[q, c] = qTs^T @ ctxT + 1 (x) cvec   (fp32r, N=512)
                es_r = esp.tile([128, C], F32R)
                simTs = []
                for _ch in range(2):
                    simT_t = ps_sim.tile([128, 512], F32, tag="sim")
                    simTs.append(simT_t)
                for ht in range(2):
                    for ch in range(2):
                        nc.tensor.matmul(
                            simTs[ch][:, :],
                            lhsT=qTs_r[:, ht, :],
                            rhs=ctxT_r[:, ht, ch * 512 : (ch + 1) * 512],
                            start=(ht == 0),
                            stop=False,
                        )
                for ch in range(2):
                    nc.tensor.matmul(
                        simTs[ch][:, :],
                        lhsT=ones_row_r[:, :],
                        rhs=cvec_r[:, ch * 512 : (ch + 1) * 512],
                        start=False,
                        stop=True,
                    )
                for ch in range(2):
                    nc.scalar.activation(
                        out=es_r[:, ch * 512 : (ch + 1) * 512],
                        in_=simTs[ch][:, :],
                        func=EXP,
                        bias=qvec_col[:, 0:1],
                        scale=1.0,
                    )

                M8w_r = smallp.tile([128, CT], F32R)
                actxa = stagp.tile([128, CT, 2, H], F32)
                cbv8 = cbvp.tile([128, CT, H], F32)
                for ct in range(CT):
                    # a-matmul with ones column: [a_unnorm | S]
                    af = ps_a.tile([128, H + 2], F32, tag="a")
                    nc.tensor.matmul(
                        af[:, :],
                        lhsT=es_r[:, ct * 128 : (ct + 1) * 128],
                        rhs=qaug_r[:, :],
                        start=True,
                        stop=True,
                    )
                    rS = smallp.tile([128, 1], F32)
                    nc.vector.reciprocal(rS[:, :], af[:, H : H + 1])
                    nc.vector.tensor_scalar_mul(actxa[:, ct, 0, :], af[:, 0:H], rS[:, :])
                    nc.gpsimd.tensor_mul(
                        actxa[:, ct, 1, :], ctx_sb[:, ct, :], actxa[:, ct, 0, :]
                    )

                    # beta max: transpose expsimT tile -> [c, q], rowmax from PSUM
                    mt = ps_tp.tile([128, 128], F32R, tag="tp")
                    nc.tensor.matmul(
                        mt[:, :],
                        lhsT=es_r[:, ct * 128 : (ct + 1) * 128],
                        rhs=ident_r[:, :],
                        start=True,
                        stop=True,
                        is_transpose=True,
                    )
                    nc.vector.reduce_max(
                        M8w_r[:, ct : ct + 1], mt[:, :].bitcast(F32), axis=X
                    )

                Sb = ps_sm.tile([1, CT], F32, tag="sm")
                nc.tensor.matmul(
                    Sb[:, :],
                    lhsT=ones_col[:, :],
                    rhs=M8w_r[:, :].bitcast(F32),
                    start=True,
                    stop=True,
                )
                Sb1 = smallp.tile([1, 1], F32)
                nc.vector.reduce_sum(Sb1[:, :], Sb[:, :], axis=X)
                rSb = smallp.tile([1, 1], F32)
                nc.vector.reciprocal(rSb[:, :], Sb1[:, :])
                bv_ps = ps_sm.tile([1, H], F32, tag="sm")
                for ct in range(CT):
                    nc.tensor.matmul(
                        bv_ps[:, :],
                        lhsT=M8w_r[:, ct : ct + 1],
                        rhs=ctx_r[:, ct, :],
                        start=(ct == 0),
                        stop=(ct == CT - 1),
                    )
                bv_r = smallp.tile([1, H], F32R)
                nc.vector.tensor_scalar_mul(bv_r[:, :], bv_ps[:, :], rSb[:, :])
                bb_ps = ps_a.tile([128, H + 2], F32, tag="a")
                nc.tensor.matmul(
                    bb_ps[:, 0:H],
                    lhsT=ones_row_r[:, :],
                    rhs=bv_r[:, :],
                    start=True,
                    stop=True,
                )
                bb_bcast = bass.AP(
                    tensor=bb_ps.tensor,
                    offset=bb_ps[:, 0:H].offset,
                    ap=[bb_ps[:, 0:H].ap[0], [0, CT], [1, H]],
                )
                nc.vector.tensor_mul(cbv8[:, :, :], ctx_sb[:, :, :], bb_bcast)
                nc.sync.dma_start(
                    out=out_d[b, :, H : 3 * H].rearrange("(ct p) h -> p ct h", p=128),
                    in_=actxa[:, :, :, :],
                )
                nc.sync.dma_start(
                    out=out_d[b, :, 3 * H : 4 * H].rearrange("(ct p) h -> p ct h", p=128),
                    in_=cbv8[:, :, :],
                )

    split_waits(nc)
    return nc


_NC = None
LAST_RESULT = None


def kernel(_trace=False, **inputs):
    global _NC, LAST_RESULT
    if _NC is None:
        _NC = build()
    context = np.ascontiguousarray(np.asarray(inputs["context"], dtype=np.float32))
    query = np.ascontiguousarray(np.asarray(inputs["query"], dtype=np.float32))
    att_w = np.ascontiguousarray(np.asarray(inputs["att_w"], dtype=np.float32))
    att_b = np.asarray(inputs["att_b"], dtype=np.float32).reshape(1)
    in_maps = [
        {
            "context": np.ascontiguousarray(context[i * BL : (i + 1) * BL]),
            "query": np.ascontiguousarray(query[i * BL : (i + 1) * BL]),
            "att_w": att_w,
            "att_b": att_b,
        }
        for i in range(NCORES)
    ]
    res = run_bass_kernel_spmd(
        _NC, in_maps, core_ids=list(range(NCORES)), trace=_trace
    )
    LAST_RESULT = res
    return np.concatenate([r["out"] for r in res.results], axis=0)
